# revision 19
# baseline (speedup 1.0000x reference)
"""Trainium2 Bass kernel for nn_Encoder (inception-conv + bidirectional stacked residual LSTM).

Sharding: 8 cores = 2 directions x 4 batch-quarters (B=32 per core).
Backward cores receive time-reversed tokens from the host (the conv stack is
time-symmetric, so every core runs a pure forward scan); the host reverses the
backward outputs back.

Per-core program (uniform SPMD, direction is data):
  - embedding via one-hot matmul, fused into conv block 1 (transposed layout
    [D-chunk parts, t])
  - 2 inception blocks: K=3/5/7 convs as tap-shifted f32r matmuls accumulated
    in PSUM, tanh (+bias) on ACT, residual add, LayerNorm via ones-matmul
    column reduction
  - 3 layer-serial LSTM scans, B=32: z kept in PSUM windows; zx (f32r,
    N=256) and bias (K=1 matmul) pre-accumulated per 8-step window; per-step
    recurrent h@Wh in bf16; gates i,f,o,g reordered so one sigmoid + one tanh
    covers them; cell state fp32.
"""

import sys

sys.path.insert(0, "/opt/trn_rl_repo")

import numpy as np
import ml_dtypes

import concourse.bass as bass
import concourse.tile as tile
from concourse import bacc, mybir
from concourse.bass_utils import run_bass_kernel_spmd

F32R = mybir.dt.float32r
BF16 = mybir.dt.bfloat16
F32 = mybir.dt.float32

B, T, D, H, V = 128, 512, 256, 256, 64
DEPTH, N_INC = 3, 2
KSIZES = (3, 5, 7)
LN_EPS = 1e-3
BC = 32          # batch rows per core
KC = D // 128    # k chunks (2)
MC = (4 * H) // 128  # m chunks of gate dim (8)
WIN = 16         # scan window (steps per PSUM group; 16x32xfp32 = 1 PSUM bank per m-chunk)
# gate reorder: reference order is i,f,g,o (split of 4H); we use i,f,o,g
GATE_PERM = np.concatenate([np.arange(0, 256), np.arange(256, 512),
                            np.arange(768, 1024), np.arange(512, 768)])


def build_nc(t_len=T, rows=BC, layers=DEPTH, blocks=N_INC, dump_x2=False):
    nwin = t_len // WIN
    nc = bacc.Bacc("TRN2", target_bir_lowering=False, debug=False,
                   enable_asserts=False, num_devices=8)

    # ---- inputs (float32r decls accept np.float32 arrays) ----
    tok = nc.dram_tensor("tok", [rows, t_len], F32, kind="ExternalInput")
    iota = nc.dram_tensor("iota", [V, 1], F32, kind="ExternalInput")
    e_w = nc.dram_tensor("e_w", [V, D], F32R, kind="ExternalInput")
    # conv weights packed per block: [blocks, 15, KC, MC2, 128] rows of taps
    # stored as [blocks, ntap_total=15, D(=cin), D(=cout)] -> we declare full
    convw = nc.dram_tensor("convw", [blocks, 15, D, D], F32R, kind="ExternalInput")
    convb = nc.dram_tensor("convb", [blocks, 3, D], F32, kind="ExternalInput")
    gamma = nc.dram_tensor("gamma", [blocks, D], F32, kind="ExternalInput")
    beta = nc.dram_tensor("beta", [blocks, D], F32, kind="ExternalInput")
    wx = nc.dram_tensor("wx", [layers, D, 4 * H], F32R, kind="ExternalInput")
    wh = nc.dram_tensor("wh", [layers, H, 4 * H], BF16, kind="ExternalInput")
    bl = nc.dram_tensor("bl", [layers, 4 * H], F32R, kind="ExternalInput")

    # ---- outputs ----
    seq = nc.dram_tensor("seq", [KC, 128, rows, t_len], F32R, kind="ExternalOutput")
    dbg = (nc.dram_tensor("dbg_x2", [KC, 128, rows, t_len], F32R,
                          kind="ExternalOutput") if dump_x2 else None)
    h_out = nc.dram_tensor("h_out", [layers, 128, KC, rows], F32, kind="ExternalOutput")
    c_out = nc.dram_tensor("c_out", [layers, 128, KC, rows], F32, kind="ExternalOutput")

    TAP_OFF = {3: 0, 5: 3, 7: 8}  # tap index offset inside the packed 15

    with tile.TileContext(nc) as tc:
        with tc.tile_pool(name="dram", bufs=1, space="DRAM") as dpool, \
             tc.tile_pool(name="consts", bufs=1) as cpool:
            # conv/scan streams in DRAM, layout [KC, 128, t, rows]
            streams = [dpool.tile([KC, 128, rows, t_len], F32R, name=f"stream{si}")
                       for si in range(layers)]  # x2 (scan input), cur1, cur2
            x2 = streams[0]

            iota_sb = cpool.tile([V, 1], F32)
            nc.sync.dma_start(out=iota_sb, in_=iota[:, :])
            e_sb = cpool.tile([V, D], F32R)
            nc.sync.dma_start(out=e_sb, in_=e_w[:, :])
            ones_sb = cpool.tile([1, WIN * rows], F32R)
            nc.vector.memset(ones_sb.bitcast(F32), 1.0)
            eps_sb = cpool.tile([1, 1], F32)
            nc.vector.memset(eps_sb, LN_EPS)
            ones128 = cpool.tile([128, 1], F32R)
            nc.vector.memset(ones128.bitcast(F32), 1.0)
            gb_sb = cpool.tile([128, blocks, 2, KC], F32)  # gamma/beta chunks
            for i in range(blocks):
                for k in range(KC):
                    nc.sync.dma_start(out=gb_sb[:, i, 0, k:k+1],
                                      in_=gamma[i, k * 128:(k + 1) * 128])
                    nc.sync.dma_start(out=gb_sb[:, i, 1, k:k+1],
                                      in_=beta[i, k * 128:(k + 1) * 128])
            cb_sb = cpool.tile([128, blocks, 3, KC], F32)  # conv bias chunks
            for i in range(blocks):
                for j in range(3):
                    for k in range(KC):
                        nc.sync.dma_start(out=cb_sb[:, i, j, k:k+1],
                                          in_=convb[i, j, k * 128:(k + 1) * 128])

            # ---------------- conv phase ----------------
            with tc.tile_pool(name="cw", bufs=1) as wpool, \
                 tc.tile_pool(name="cd", bufs=3, space="DRAM") as dspool, \
                 tc.tile_pool(name="cx", bufs=2) as xpool, \
                 tc.tile_pool(name="cf", bufs=2) as fpool, \
                 tc.tile_pool(name="ct", bufs=2) as tpool, \
                 tc.tile_pool(name="cs", bufs=2) as spool, \
                 tc.tile_pool(name="cps", bufs=2, space="PSUM") as cps, \
                 tc.tile_pool(name="sps", bufs=2, space="PSUM") as sps:
                w_sb = wpool.tile([128, blocks, 15, KC, KC, 128], F32R)
                for i in range(blocks):
                    for tap in range(15):
                        for k in range(KC):
                            # convw[i, tap, cin, cout]: lhsT slice [cin128, cout128]
                            for m in range(KC):
                                nc.sync.dma_start(
                                    out=w_sb[:, i, tap, k, m, :],
                                    in_=convw[i, tap, k * 128:(k + 1) * 128,
                                              m * 128:(m + 1) * 128])

                for b in range(rows):
                    # one-hot: [V, t]
                    ohf = xpool.tile([V, t_len], F32, tag="ohf")
                    tok_b = bass.AP(tensor=tok.ap().tensor, offset=b * t_len,
                                    ap=[[0, V], [1, t_len]])
                    nc.sync.dma_start(out=ohf, in_=tok_b)
                    oh = xpool.tile([V, t_len], F32R, tag="oh")
                    nc.vector.tensor_scalar(
                        out=oh, in0=ohf, scalar1=iota_sb[:V, :],
                        scalar2=None, op0=mybir.AluOpType.is_equal)

                    # xpad [128, KC, t+6] with zero edges
                    xpad = xpool.tile([128, KC, t_len + 6], F32R, tag="xpad")
                    nc.vector.memset(xpad[:, :, 0:3].bitcast(F32), 0.0)
                    nc.vector.memset(xpad[:, :, t_len + 3:t_len + 6].bitcast(F32), 0.0)
                    for m in range(KC):
                        pe = cps.tile([128, t_len], F32, tag="embps")
                        nc.tensor.matmul(pe[:, :], e_sb[:, m * 128:(m + 1) * 128],
                                         oh[:, :], start=True, stop=True)
                        nc.scalar.activation(xpad[:, m, 3:3 + t_len], pe[:, :],
                                             mybir.ActivationFunctionType.Copy)

                    for i in range(blocks):
                        feats = fpool.tile([128, KC, t_len], F32R, tag="feats")
                        for j, ks in enumerate(KSIZES):
                            for m in range(KC):
                                pj = cps.tile([128, t_len], F32, tag="convps")
                                nmm = ks * KC
                                cnt = 0
                                for tap in range(ks):
                                    off = 3 + tap - ks // 2
                                    for k in range(KC):
                                        nc.tensor.matmul(
                                            pj[:, :],
                                            w_sb[:, i, TAP_OFF[ks] + tap, k, m, :],
                                            xpad[:, k, off:off + t_len],
                                            start=(cnt == 0), stop=(cnt == nmm - 1))
                                        cnt += 1
                                # tanh(conv + bias) -> accumulate into feats
                                if j == 0:
                                    nc.scalar.activation(
                                        feats[:, m, :], pj[:, :],
                                        mybir.ActivationFunctionType.Tanh,
                                        bias=cb_sb[:, i, j, m:m+1])
                                else:
                                    tj = tpool.tile([128, t_len], F32R, tag="tj")
                                    nc.scalar.activation(
                                        tj[:, :], pj[:, :],
                                        mybir.ActivationFunctionType.Tanh,
                                        bias=cb_sb[:, i, j, m:m+1])
                                    nc.vector.tensor_add(feats[:, m, :],
                                                         feats[:, m, :], tj[:, :])
                        # residual add x
                        for m in range(KC):
                            nc.vector.tensor_add(feats[:, m, :], feats[:, m, :],
                                                 xpad[:, m, 3:3 + t_len])
                        # LayerNorm over D via ones-matmul
                        mu_ps = sps.tile([1, t_len], F32, tag="mups")
                        sq = tpool.tile([128, t_len], F32R, tag="sq")
                        sq_ps = sps.tile([1, t_len], F32, tag="sqps")
                        for m in range(KC):
                            nc.tensor.matmul(mu_ps[:, :], ones128[:, :],
                                             feats[:, m, :], start=(m == 0),
                                             stop=(m == KC - 1))
                        for m in range(KC):
                            nc.scalar.square(sq[:, :], feats[:, m, :])
                            nc.tensor.matmul(sq_ps[:, :], ones128[:, :], sq[:, :],
                                             start=(m == 0), stop=(m == KC - 1))
                        stat = spool.tile([1, 4, t_len], F32, tag="stat")
                        nc.vector.tensor_scalar_mul(stat[:, 0, :], mu_ps[:, :],
                                                    1.0 / D)   # mean
                        nc.vector.tensor_scalar_mul(stat[:, 1, :], sq_ps[:, :],
                                                    1.0 / D)   # E[x^2]
                        nc.vector.tensor_mul(stat[:, 2, :], stat[:, 0, :],
                                             stat[:, 0, :])    # mean^2
                        nc.vector.tensor_sub(stat[:, 1, :], stat[:, 1, :],
                                             stat[:, 2, :])    # var
                        nc.scalar.activation(stat[:, 1, :], stat[:, 1, :],
                                             mybir.ActivationFunctionType.Sqrt,
                                             bias=eps_sb[:, :])
                        nc.vector.reciprocal(stat[:, 2, :], stat[:, 1, :])  # rstd
                        # broadcast mean/rstd across partitions (via DRAM)
                        stat_d = dspool.tile([2, t_len], F32, tag="stat_d")
                        nc.sync.dma_start(out=stat_d[0:1, :], in_=stat[:, 0, :])
                        nc.sync.dma_start(out=stat_d[1:2, :], in_=stat[:, 2, :])
                        bc2 = spool.tile([128, 2, t_len], F32, tag="bc2")
                        sd_ap = bass.AP(tensor=stat_d.tensor, offset=stat_d.offset,
                                        ap=[[0, 128], [t_len, 2], [1, t_len]])
                        nc.sync.dma_start(out=bc2[:, :, :], in_=sd_ap)
                        is_last = (i == blocks - 1)
                        for m in range(KC):
                            nc.vector.scalar_tensor_tensor(
                                out=feats[:, m, :], in0=feats[:, m, :],
                                scalar=1.0, in1=bc2[:, 0, :],
                                op0=mybir.AluOpType.mult,
                                op1=mybir.AluOpType.subtract)
                            nc.vector.tensor_mul(feats[:, m, :], feats[:, m, :],
                                                 bc2[:, 1, :])
                            dst = xpad[:, m, 3:3 + t_len]
                            nc.vector.tensor_scalar(
                                out=dst, in0=feats[:, m, :],
                                scalar1=gb_sb[:, i, 0, m:m+1], scalar2=gb_sb[:, i, 1, m:m+1],
                                op0=mybir.AluOpType.mult, op1=mybir.AluOpType.add)
                        if is_last:
                            for m in range(KC):
                                nc.sync.dma_start(out=x2[m, :, b, :],
                                                  in_=xpad[:, m, 3:3 + t_len])

            if dump_x2:
                for k in range(KC):
                    n_el = 128 * rows * t_len
                    src_ap = bass.AP(tensor=x2.tensor, offset=x2.offset + k * n_el,
                                     ap=[[rows * t_len, 128], [1, rows * t_len]])
                    dst_ap = bass.AP(tensor=dbg.ap().tensor, offset=k * n_el,
                                     ap=[[rows * t_len, 128], [1, rows * t_len]])
                    nc.sync.dma_start(out=dst_ap, in_=src_ap)

            # ---------------- scan phase ----------------
            with tc.tile_pool(name="sw", bufs=1) as swpool, \
                 tc.tile_pool(name="st", bufs=3) as stpool, \
                 tc.tile_pool(name="sst", bufs=1) as sstpool, \
                 tc.tile_pool(name="zps", bufs=1, space="PSUM") as zps:
                wx_sb = swpool.tile([128, layers, KC, MC, 128], F32R)
                wh_sb = swpool.tile([128, layers, KC, MC, 128], BF16)
                for l in range(layers):
                    for k in range(KC):
                        for m in range(MC):
                            nc.sync.dma_start(
                                out=wx_sb[:, l, k, m, :],
                                in_=wx[l, k * 128:(k + 1) * 128,
                                       m * 128:(m + 1) * 128])
                            nc.sync.dma_start(
                                out=wh_sb[:, l, k, m, :],
                                in_=wh[l, k * 128:(k + 1) * 128,
                                       m * 128:(m + 1) * 128])
                bl_sb = swpool.tile([1, layers, MC, 128], F32R)
                for l in range(layers):
                    nc.sync.dma_start(out=bl_sb[0:1, l, :, :], in_=bl[l:l + 1, :])

                for l in range(layers):
                    src = streams[l]
                    dst = seq.ap() if l == layers - 1 else streams[l + 1]
                    cst = sstpool.tile([128, KC, rows], F32, tag="c_state")
                    nc.vector.memset(cst, 0.0)
                    h_prev = sstpool.tile([128, KC, rows], BF16, tag="h_init")
                    nc.vector.memset(h_prev, 0.0)

                    for w in range(nwin):
                        t0 = w * WIN
                        cin = stpool.tile([128, KC, WIN, rows], F32R, tag="cin")
                        for k in range(KC):
                            for s in range(WIN):
                                src_ap = bass.AP(
                                    tensor=src.tensor,
                                    offset=src.offset + k * 128 * rows * t_len
                                    + (t0 + s),
                                    ap=[[rows * t_len, 128], [t_len, rows]])
                                nc.sync.dma_start(out=cin[:, k, s, :], in_=src_ap)
                        zw = zps.tile([128, MC, WIN, rows], F32, tag="zw")
                        for m in range(MC):
                            for k in range(KC):
                                nc.tensor.matmul(
                                    zw[:, m, :, :], wx_sb[:, l, k, m, :],
                                    cin[:, k, :, :], start=(k == 0), stop=False,
                                    skip_group_check=True)
                            nc.tensor.matmul(
                                zw[:, m, :, :], bl_sb[0:1, l, m, :], ones_sb[:, :],
                                start=False, stop=False, skip_group_check=True)
                        cout = stpool.tile([128, KC, WIN, rows], F32R, tag="cout")
                        for s in range(WIN):
                            for m in range(MC):
                                for k in range(KC):
                                    nc.tensor.matmul(
                                        zw[:, m, s, :], wh_sb[:, l, k, m, :],
                                        h_prev[:, k, :], start=False,
                                        stop=(k == KC - 1), skip_group_check=True)
                            sig = stpool.tile([128, 6, rows], BF16, tag="sig")
                            nc.scalar.activation(sig[:, :, :], zw[:, 0:6, s, :],
                                                 mybir.ActivationFunctionType.Sigmoid)
                            tg = stpool.tile([128, KC, rows], BF16, tag="tg")
                            nc.scalar.activation(tg[:, :, :], zw[:, 6:8, s, :],
                                                 mybir.ActivationFunctionType.Tanh)
                            u = stpool.tile([128, KC, rows], F32, tag="u")
                            nc.gpsimd.tensor_mul(u[:, :, :], sig[:, 0:2, :],
                                                 tg[:, :, :])
                            mc_t = stpool.tile([128, KC, rows], F32, tag="mc")
                            nc.vector.tensor_mul(mc_t[:, :, :], cst[:, :, :],
                                                 sig[:, 2:4, :])
                            nc.vector.tensor_add(cst[:, :, :], mc_t[:, :, :],
                                                 u[:, :, :])
                            tc_t = stpool.tile([128, KC, rows], BF16, tag="tc")
                            nc.scalar.activation(tc_t[:, :, :], cst[:, :, :],
                                                 mybir.ActivationFunctionType.Tanh)
                            h_new = stpool.tile([128, KC, rows], BF16, tag="h")
                            nc.vector.tensor_mul(h_new[:, :, :], sig[:, 4:6, :],
                                                 tc_t[:, :, :])
                            nc.vector.tensor_add(cout[:, :, s, :], cin[:, :, s, :],
                                                 h_new[:, :, :])
                            h_prev = h_new
                        for k in range(KC):
                            for s in range(WIN):
                                dst_ap = bass.AP(
                                    tensor=dst.tensor,
                                    offset=dst.offset + k * 128 * rows * t_len
                                    + (t0 + s),
                                    ap=[[rows * t_len, 128], [t_len, rows]])
                                nc.sync.dma_start(out=dst_ap, in_=cout[:, k, s, :])
                    # final states
                    hf = stpool.tile([128, KC, rows], F32, tag="hf")
                    nc.vector.tensor_copy(hf[:, :, :], h_prev[:, :, :])
                    nc.sync.dma_start(out=h_out[l, :, :, :], in_=hf[:, :, :])
                    nc.sync.dma_start(out=c_out[l, :, :, :], in_=cst[:, :, :])

    nc.compile()
    return nc


_NC_CACHE = {}


def _get_nc():
    if "nc" not in _NC_CACHE:
        _NC_CACHE["nc"] = build_nc()
    return _NC_CACHE["nc"]


def kernel(tokens, E, W3, W5, W7, bconv, gamma, beta, Wx, Wh, blstm):
    tokens = np.asarray(tokens)
    E = np.asarray(E, np.float32)
    W3 = np.asarray(W3, np.float32)
    W5 = np.asarray(W5, np.float32)
    W7 = np.asarray(W7, np.float32)
    bconv = np.asarray(bconv, np.float32)
    gamma_np = np.asarray(gamma, np.float32)
    beta_np = np.asarray(beta, np.float32)
    Wx = np.asarray(Wx, np.float32)
    Wh = np.asarray(Wh, np.float32)
    blstm = np.asarray(blstm, np.float32)

    # packed conv weights [blocks, 15, cin, cout] in tap order K3,K5,K7.
    # Backward cores run on time-reversed tokens; reverse(conv(x, W)) =
    # conv(reverse(x), flip(W, taps)), so they get tap-flipped weights.
    convw_f = np.concatenate([W3, W5, W7], axis=1)
    convw_b = np.concatenate([W3[:, ::-1], W5[:, ::-1], W7[:, ::-1]], axis=1)
    convw_b = np.ascontiguousarray(convw_b)
    iota = np.arange(V, dtype=np.float32).reshape(V, 1)

    in_maps = []
    for c in range(8):
        d = c // 4            # 0 = forward, 1 = backward
        s = c % 4
        rows = slice(32 * s, 32 * s + 32)
        tk = tokens[rows].astype(np.float32)
        if d == 1:
            tk = tk[:, ::-1]
        in_maps.append({
            "tok": np.ascontiguousarray(tk),
            "iota": iota,
            "e_w": E,
            "convw": convw_f if d == 0 else convw_b,
            "convb": bconv,
            "gamma": gamma_np,
            "beta": beta_np,
            "wx": np.ascontiguousarray(Wx[d][:, :, GATE_PERM]),
            "wh": np.ascontiguousarray(Wh[d][:, :, GATE_PERM]).astype(
                ml_dtypes.bfloat16),
            "bl": np.ascontiguousarray(blstm[d][:, GATE_PERM]),
        })

    global _last_in_maps
    _last_in_maps = in_maps
    nc = _get_nc()
    res = run_bass_kernel_spmd(nc, in_maps, core_ids=list(range(8)))

    out = np.zeros((B, T, 2 * H), np.float32)
    states_h = np.zeros((2, DEPTH, B, H), np.float32)
    states_c = np.zeros((2, DEPTH, B, H), np.float32)
    for c in range(8):
        d, s = c // 4, c % 4
        rows = slice(32 * s, 32 * s + 32)
        r = res.results[c]
        sq = r["seq"].reshape(D, BC, T).transpose(1, 2, 0)  # [b, t, feat]
        hs = r["h_out"].transpose(0, 2, 1, 3).reshape(DEPTH, D, BC).transpose(0, 2, 1)  # [l, b, feat]
        cs = r["c_out"].transpose(0, 2, 1, 3).reshape(DEPTH, D, BC).transpose(0, 2, 1)
        if d == 0:
            out[rows, :, :H] = sq
        else:
            out[rows, :, H:] = sq[:, ::-1, :]
        states_h[d, :, rows, :] = hs
        states_c[d, :, rows, :] = cs
    return out, states_h, states_c


# revision 23
# speedup vs baseline: 2.1583x; 2.1583x over previous
"""Trainium2 Bass kernel for nn_Encoder (inception-conv + bidirectional stacked residual LSTM).

Sharding: 8 cores = 2 directions x 4 batch-quarters (B=32 per core).
Backward cores receive time-reversed tokens from the host (the conv stack is
time-symmetric, so every core runs a pure forward scan); the host reverses the
backward outputs back.

Per-core program (uniform SPMD, direction is data):
  - embedding via one-hot matmul, fused into conv block 1 (transposed layout
    [D-chunk parts, t])
  - 2 inception blocks: K=3/5/7 convs as tap-shifted f32r matmuls accumulated
    in PSUM, tanh (+bias) on ACT, residual add, LayerNorm via ones-matmul
    column reduction
  - 3 layer-serial LSTM scans, B=32: z kept in PSUM windows; zx (f32r,
    N=256) and bias (K=1 matmul) pre-accumulated per 8-step window; per-step
    recurrent h@Wh in bf16; gates i,f,o,g reordered so one sigmoid + one tanh
    covers them; cell state fp32.
"""

import sys

sys.path.insert(0, "/opt/trn_rl_repo")

import numpy as np
import ml_dtypes

import concourse.bass as bass
import concourse.tile as tile
from concourse import bacc, mybir
from concourse.bass_utils import run_bass_kernel_spmd

F32R = mybir.dt.float32r
BF16 = mybir.dt.bfloat16
F32 = mybir.dt.float32

B, T, D, H, V = 128, 512, 256, 256, 64
DEPTH, N_INC = 3, 2
KSIZES = (3, 5, 7)
LN_EPS = 1e-3
BC = 32          # batch rows per core
KC = D // 128    # k chunks (2)
MC = (4 * H) // 128  # m chunks of gate dim (8)
WIN = 16         # scan window (steps per PSUM group; 16x32xfp32 = 1 PSUM bank per m-chunk)
# gate reorder: reference order is i,f,g,o (split of 4H); we use i,f,o,g
GATE_PERM = np.concatenate([np.arange(0, 256), np.arange(256, 512),
                            np.arange(768, 1024), np.arange(512, 768)])


def build_nc(t_len=T, rows=BC, layers=DEPTH, blocks=N_INC, dump_x2=False, do_conv=True):
    nwin = t_len // WIN
    nc = bacc.Bacc("TRN2", target_bir_lowering=False, debug=False,
                   enable_asserts=False, num_devices=8)

    # ---- inputs (float32r decls accept np.float32 arrays) ----
    tok = nc.dram_tensor("tok", [rows, t_len], F32, kind="ExternalInput")
    iota = nc.dram_tensor("iota", [V, 1], F32, kind="ExternalInput")
    e_w = nc.dram_tensor("e_w", [V, D], F32R, kind="ExternalInput")
    # conv weights packed per block: [blocks, 15, KC, MC2, 128] rows of taps
    # stored as [blocks, ntap_total=15, D(=cin), D(=cout)] -> we declare full
    convw = nc.dram_tensor("convw", [blocks, 15, D, D], F32R, kind="ExternalInput")
    convb = nc.dram_tensor("convb", [blocks, 3, D], F32, kind="ExternalInput")
    gamma = nc.dram_tensor("gamma", [blocks, D], F32, kind="ExternalInput")
    beta = nc.dram_tensor("beta", [blocks, D], F32, kind="ExternalInput")
    wx = nc.dram_tensor("wx", [layers, D, 4 * H], F32R, kind="ExternalInput")
    wh = nc.dram_tensor("wh", [layers, H, 4 * H], BF16, kind="ExternalInput")
    bl = nc.dram_tensor("bl", [layers, 4 * H], F32R, kind="ExternalInput")

    # ---- outputs ----
    seq = nc.dram_tensor("seq", [KC, 128, rows, t_len], F32R, kind="ExternalOutput")
    dbg = (nc.dram_tensor("dbg_x2", [KC, 128, rows, t_len], F32R,
                          kind="ExternalOutput") if dump_x2 else None)
    h_out = nc.dram_tensor("h_out", [layers, 128, KC, rows], F32, kind="ExternalOutput")
    c_out = nc.dram_tensor("c_out", [layers, 128, KC, rows], F32, kind="ExternalOutput")

    TAP_OFF = {3: 0, 5: 3, 7: 8}  # tap index offset inside the packed 15

    with tile.TileContext(nc) as tc:
        with tc.tile_pool(name="dram", bufs=1, space="DRAM") as dpool, \
             tc.tile_pool(name="consts", bufs=1) as cpool:
            # conv/scan streams in DRAM, layout [KC, 128, t, rows]
            streams = [dpool.tile([KC, 128, rows, t_len], F32R, name=f"stream{si}")
                       for si in range(layers)]  # x2 (scan input), cur1, cur2
            x2 = streams[0]

            iota_sb = cpool.tile([V, 1], F32)
            nc.sync.dma_start(out=iota_sb, in_=iota[:, :])
            e_sb = cpool.tile([V, D], F32R)
            nc.sync.dma_start(out=e_sb, in_=e_w[:, :])
            ones_sb = cpool.tile([1, WIN * rows], F32R)
            nc.vector.memset(ones_sb.bitcast(F32), 1.0)
            eps_sb = cpool.tile([1, 1], F32)
            nc.vector.memset(eps_sb, LN_EPS)
            ones128 = cpool.tile([128, 1], F32R)
            nc.vector.memset(ones128.bitcast(F32), 1.0)
            gb_sb = cpool.tile([128, blocks, 2, KC], F32)  # gamma/beta chunks
            for i in range(blocks):
                for k in range(KC):
                    nc.sync.dma_start(out=gb_sb[:, i, 0, k:k+1],
                                      in_=gamma[i, k * 128:(k + 1) * 128])
                    nc.sync.dma_start(out=gb_sb[:, i, 1, k:k+1],
                                      in_=beta[i, k * 128:(k + 1) * 128])
            cb_sb = cpool.tile([128, blocks, 3, KC], F32)  # conv bias chunks
            for i in range(blocks):
                for j in range(3):
                    for k in range(KC):
                        nc.sync.dma_start(out=cb_sb[:, i, j, k:k+1],
                                          in_=convb[i, j, k * 128:(k + 1) * 128])

            # ---------------- conv phase ----------------
            with tc.tile_pool(name="cw", bufs=1) as wpool, \
                 tc.tile_pool(name="cd", bufs=3, space="DRAM") as dspool, \
                 tc.tile_pool(name="cx", bufs=2) as xpool, \
                 tc.tile_pool(name="cf", bufs=2) as fpool, \
                 tc.tile_pool(name="ct", bufs=2) as tpool, \
                 tc.tile_pool(name="cs", bufs=2) as spool, \
                 tc.tile_pool(name="cps", bufs=2, space="PSUM") as cps, \
                 tc.tile_pool(name="sps", bufs=2, space="PSUM") as sps:
                w_sb = wpool.tile([128, blocks, 15, KC, KC, 128], F32R)
                for i in range(blocks):
                    for tap in range(15):
                        for k in range(KC):
                            # convw[i, tap, cin, cout]: lhsT slice [cin128, cout128]
                            for m in range(KC):
                                nc.sync.dma_start(
                                    out=w_sb[:, i, tap, k, m, :],
                                    in_=convw[i, tap, k * 128:(k + 1) * 128,
                                              m * 128:(m + 1) * 128])

                for b in range(rows):
                    # one-hot: [V, t]
                    ohf = xpool.tile([V, t_len], F32, tag="ohf")
                    tok_b = bass.AP(tensor=tok.ap().tensor, offset=b * t_len,
                                    ap=[[0, V], [1, t_len]])
                    nc.sync.dma_start(out=ohf, in_=tok_b)
                    oh = xpool.tile([V, t_len], F32R, tag="oh")
                    nc.vector.tensor_scalar(
                        out=oh, in0=ohf, scalar1=iota_sb[:V, :],
                        scalar2=None, op0=mybir.AluOpType.is_equal)

                    # xpad [128, KC, t+6] with zero edges
                    xpad = xpool.tile([128, KC, t_len + 6], F32R, tag="xpad")
                    nc.vector.memset(xpad[:, :, 0:3].bitcast(F32), 0.0)
                    nc.vector.memset(xpad[:, :, t_len + 3:t_len + 6].bitcast(F32), 0.0)
                    for m in range(KC):
                        pe = cps.tile([128, t_len], F32, tag="embps")
                        nc.tensor.matmul(pe[:, :], e_sb[:, m * 128:(m + 1) * 128],
                                         oh[:, :], start=True, stop=True)
                        nc.scalar.activation(xpad[:, m, 3:3 + t_len], pe[:, :],
                                             mybir.ActivationFunctionType.Copy)

                    if not do_conv:
                        for m in range(KC):
                            nc.sync.dma_start(out=x2[m, :, b, :],
                                              in_=xpad[:, m, 3:3 + t_len])
                    for i in (range(blocks) if do_conv else ()):
                        feats = fpool.tile([128, KC, t_len], F32R, tag="feats")
                        for j, ks in enumerate(KSIZES):
                            for m in range(KC):
                                pj = cps.tile([128, t_len], F32, tag="convps")
                                nmm = ks * KC
                                cnt = 0
                                for tap in range(ks):
                                    off = 3 + tap - ks // 2
                                    for k in range(KC):
                                        nc.tensor.matmul(
                                            pj[:, :],
                                            w_sb[:, i, TAP_OFF[ks] + tap, k, m, :],
                                            xpad[:, k, off:off + t_len],
                                            start=(cnt == 0), stop=(cnt == nmm - 1))
                                        cnt += 1
                                # tanh(conv + bias) -> accumulate into feats
                                if j == 0:
                                    nc.scalar.activation(
                                        feats[:, m, :], pj[:, :],
                                        mybir.ActivationFunctionType.Tanh,
                                        bias=cb_sb[:, i, j, m:m+1])
                                else:
                                    tj = tpool.tile([128, t_len], F32R, tag="tj")
                                    nc.scalar.activation(
                                        tj[:, :], pj[:, :],
                                        mybir.ActivationFunctionType.Tanh,
                                        bias=cb_sb[:, i, j, m:m+1])
                                    nc.vector.tensor_add(feats[:, m, :],
                                                         feats[:, m, :], tj[:, :])
                        # residual add x
                        for m in range(KC):
                            nc.vector.tensor_add(feats[:, m, :], feats[:, m, :],
                                                 xpad[:, m, 3:3 + t_len])
                        # LayerNorm over D via ones-matmul
                        mu_ps = sps.tile([1, t_len], F32, tag="mups")
                        sq = tpool.tile([128, t_len], F32R, tag="sq")
                        sq_ps = sps.tile([1, t_len], F32, tag="sqps")
                        for m in range(KC):
                            nc.tensor.matmul(mu_ps[:, :], ones128[:, :],
                                             feats[:, m, :], start=(m == 0),
                                             stop=(m == KC - 1))
                        for m in range(KC):
                            nc.scalar.square(sq[:, :], feats[:, m, :])
                            nc.tensor.matmul(sq_ps[:, :], ones128[:, :], sq[:, :],
                                             start=(m == 0), stop=(m == KC - 1))
                        stat = spool.tile([1, 4, t_len], F32, tag="stat")
                        nc.vector.tensor_scalar_mul(stat[:, 0, :], mu_ps[:, :],
                                                    1.0 / D)   # mean
                        nc.vector.tensor_scalar_mul(stat[:, 1, :], sq_ps[:, :],
                                                    1.0 / D)   # E[x^2]
                        nc.vector.tensor_mul(stat[:, 2, :], stat[:, 0, :],
                                             stat[:, 0, :])    # mean^2
                        nc.vector.tensor_sub(stat[:, 1, :], stat[:, 1, :],
                                             stat[:, 2, :])    # var
                        nc.scalar.activation(stat[:, 1, :], stat[:, 1, :],
                                             mybir.ActivationFunctionType.Sqrt,
                                             bias=eps_sb[:, :])
                        nc.vector.reciprocal(stat[:, 2, :], stat[:, 1, :])  # rstd
                        # broadcast mean/rstd across partitions (via DRAM)
                        stat_d = dspool.tile([2, t_len], F32, tag="stat_d")
                        nc.sync.dma_start(out=stat_d[0:1, :], in_=stat[:, 0, :])
                        nc.sync.dma_start(out=stat_d[1:2, :], in_=stat[:, 2, :])
                        bc2 = spool.tile([128, 2, t_len], F32, tag="bc2")
                        sd_ap = bass.AP(tensor=stat_d.tensor, offset=stat_d.offset,
                                        ap=[[0, 128], [t_len, 2], [1, t_len]])
                        nc.sync.dma_start(out=bc2[:, :, :], in_=sd_ap)
                        is_last = (i == blocks - 1)
                        for m in range(KC):
                            nc.vector.scalar_tensor_tensor(
                                out=feats[:, m, :], in0=feats[:, m, :],
                                scalar=1.0, in1=bc2[:, 0, :],
                                op0=mybir.AluOpType.mult,
                                op1=mybir.AluOpType.subtract)
                            nc.vector.tensor_mul(feats[:, m, :], feats[:, m, :],
                                                 bc2[:, 1, :])
                            dst = xpad[:, m, 3:3 + t_len]
                            nc.vector.tensor_scalar(
                                out=dst, in0=feats[:, m, :],
                                scalar1=gb_sb[:, i, 0, m:m+1], scalar2=gb_sb[:, i, 1, m:m+1],
                                op0=mybir.AluOpType.mult, op1=mybir.AluOpType.add)
                        if is_last:
                            for m in range(KC):
                                nc.sync.dma_start(out=x2[m, :, b, :],
                                                  in_=xpad[:, m, 3:3 + t_len])

            if dump_x2:
                for k in range(KC):
                    n_el = 128 * rows * t_len
                    src_ap = bass.AP(tensor=x2.tensor, offset=x2.offset + k * n_el,
                                     ap=[[rows * t_len, 128], [1, rows * t_len]])
                    dst_ap = bass.AP(tensor=dbg.ap().tensor, offset=k * n_el,
                                     ap=[[rows * t_len, 128], [1, rows * t_len]])
                    nc.sync.dma_start(out=dst_ap, in_=src_ap)

            # ---------------- scan phase ----------------
            with tc.tile_pool(name="sw", bufs=1) as swpool, \
                 tc.tile_pool(name="st", bufs=3) as stpool, \
                 tc.tile_pool(name="sst", bufs=1) as sstpool, \
                 tc.tile_pool(name="zps", bufs=1, space="PSUM") as zps:
                wx_sb = swpool.tile([128, layers, KC, MC, 128], F32R)
                wh_sb = swpool.tile([128, layers, KC, MC, 128], BF16)
                for l in range(layers):
                    for k in range(KC):
                        for m in range(MC):
                            nc.sync.dma_start(
                                out=wx_sb[:, l, k, m, :],
                                in_=wx[l, k * 128:(k + 1) * 128,
                                       m * 128:(m + 1) * 128])
                            nc.sync.dma_start(
                                out=wh_sb[:, l, k, m, :],
                                in_=wh[l, k * 128:(k + 1) * 128,
                                       m * 128:(m + 1) * 128])
                bl_sb = swpool.tile([1, layers, MC, 128], F32R)
                for l in range(layers):
                    nc.sync.dma_start(out=bl_sb[0:1, l, :, :], in_=bl[l:l + 1, :])

                for l in range(layers):
                    src = streams[l]
                    dst = seq.ap() if l == layers - 1 else streams[l + 1]
                    cst = sstpool.tile([128, KC, rows], F32, tag="c_state")
                    nc.vector.memset(cst, 0.0)
                    h_prev = sstpool.tile([128, KC, rows], BF16, tag="h_init")
                    nc.vector.memset(h_prev, 0.0)

                    for w in range(nwin):
                        t0 = w * WIN
                        cin = stpool.tile([128, KC, WIN, rows], F32R, tag="cin")
                        for k in range(KC):
                            for s in range(WIN):
                                src_ap = bass.AP(
                                    tensor=src.tensor,
                                    offset=src.offset + k * 128 * rows * t_len
                                    + (t0 + s),
                                    ap=[[rows * t_len, 128], [t_len, rows]])
                                nc.sync.dma_start(out=cin[:, k, s, :], in_=src_ap)
                        zw = zps.tile([128, MC, WIN, rows], F32, tag="zw")
                        for m in range(MC):
                            for k in range(KC):
                                nc.tensor.matmul(
                                    zw[:, m, :, :], wx_sb[:, l, k, m, :],
                                    cin[:, k, :, :], start=(k == 0), stop=False,
                                    skip_group_check=True)
                            nc.tensor.matmul(
                                zw[:, m, :, :], bl_sb[0:1, l, m, :], ones_sb[:, :],
                                start=False, stop=False, skip_group_check=True)
                        cout = stpool.tile([128, KC, WIN, rows], F32R, tag="cout")
                        for s in range(WIN):
                            for m in range(MC):
                                for k in range(KC):
                                    nc.tensor.matmul(
                                        zw[:, m, s, :], wh_sb[:, l, k, m, :],
                                        h_prev[:, k, :], start=False,
                                        stop=(k == KC - 1), skip_group_check=True)
                            sig = stpool.tile([128, 6, rows], BF16, tag="sig")
                            nc.scalar.activation(sig[:, :, :], zw[:, 0:6, s, :],
                                                 mybir.ActivationFunctionType.Sigmoid)
                            tg = stpool.tile([128, KC, rows], BF16, tag="tg")
                            nc.scalar.activation(tg[:, :, :], zw[:, 6:8, s, :],
                                                 mybir.ActivationFunctionType.Tanh)
                            u = stpool.tile([128, KC, rows], F32, tag="u")
                            nc.gpsimd.tensor_mul(u[:, :, :], sig[:, 0:2, :],
                                                 tg[:, :, :])
                            mc_t = stpool.tile([128, KC, rows], F32, tag="mc")
                            nc.vector.tensor_mul(mc_t[:, :, :], cst[:, :, :],
                                                 sig[:, 2:4, :])
                            nc.vector.tensor_add(cst[:, :, :], mc_t[:, :, :],
                                                 u[:, :, :])
                            tc_t = stpool.tile([128, KC, rows], BF16, tag="tc")
                            nc.scalar.activation(tc_t[:, :, :], cst[:, :, :],
                                                 mybir.ActivationFunctionType.Tanh)
                            h_new = stpool.tile([128, KC, rows], BF16, tag="h")
                            nc.vector.tensor_mul(h_new[:, :, :], sig[:, 4:6, :],
                                                 tc_t[:, :, :])
                            nc.vector.tensor_add(cout[:, :, s, :], cin[:, :, s, :],
                                                 h_new[:, :, :])
                            h_prev = h_new
                        for k in range(KC):
                            for s in range(WIN):
                                dst_ap = bass.AP(
                                    tensor=dst.tensor,
                                    offset=dst.offset + k * 128 * rows * t_len
                                    + (t0 + s),
                                    ap=[[rows * t_len, 128], [t_len, rows]])
                                nc.sync.dma_start(out=dst_ap, in_=cout[:, k, s, :])
                    # final states
                    hf = stpool.tile([128, KC, rows], F32, tag="hf")
                    nc.vector.tensor_copy(hf[:, :, :], h_prev[:, :, :])
                    nc.sync.dma_start(out=h_out[l, :, :, :], in_=hf[:, :, :])
                    nc.sync.dma_start(out=c_out[l, :, :, :], in_=cst[:, :, :])

    nc.compile()
    return nc


_NC_CACHE = {}


def _get_nc():
    if "nc" not in _NC_CACHE:
        _NC_CACHE["nc"] = build_nc()
    return _NC_CACHE["nc"]


def make_in_maps(tokens, E, W3, W5, W7, bconv, gamma, beta, Wx, Wh, blstm):
    tokens = np.asarray(tokens)
    E = np.asarray(E, np.float32)
    W3 = np.asarray(W3, np.float32)
    W5 = np.asarray(W5, np.float32)
    W7 = np.asarray(W7, np.float32)
    bconv = np.asarray(bconv, np.float32)
    gamma_np = np.asarray(gamma, np.float32)
    beta_np = np.asarray(beta, np.float32)
    Wx = np.asarray(Wx, np.float32)
    Wh = np.asarray(Wh, np.float32)
    blstm = np.asarray(blstm, np.float32)

    # packed conv weights [blocks, 15, cin, cout] in tap order K3,K5,K7.
    # Backward cores run on time-reversed tokens; reverse(conv(x, W)) =
    # conv(reverse(x), flip(W, taps)), so they get tap-flipped weights.
    convw_f = np.concatenate([W3, W5, W7], axis=1)
    convw_b = np.concatenate([W3[:, ::-1], W5[:, ::-1], W7[:, ::-1]], axis=1)
    convw_b = np.ascontiguousarray(convw_b)
    iota = np.arange(V, dtype=np.float32).reshape(V, 1)

    in_maps = []
    for c in range(8):
        d = c // 4            # 0 = forward, 1 = backward
        s = c % 4
        rows = slice(32 * s, 32 * s + 32)
        tk = tokens[rows].astype(np.float32)
        if d == 1:
            tk = tk[:, ::-1]
        in_maps.append({
            "tok": np.ascontiguousarray(tk),
            "iota": iota,
            "e_w": E,
            "convw": convw_f if d == 0 else convw_b,
            "convb": bconv,
            "gamma": gamma_np,
            "beta": beta_np,
            "wx": np.ascontiguousarray(Wx[d][:, :, GATE_PERM]),
            "wh": np.ascontiguousarray(Wh[d][:, :, GATE_PERM]).astype(
                ml_dtypes.bfloat16),
            "bl": np.ascontiguousarray(blstm[d][:, GATE_PERM]),
        })

    return in_maps


def kernel(**inputs):
    global _last_in_maps
    in_maps = make_in_maps(**inputs)
    _last_in_maps = in_maps
    nc = _get_nc()
    res = run_bass_kernel_spmd(nc, in_maps, core_ids=list(range(8)))

    out = np.zeros((B, T, 2 * H), np.float32)
    states_h = np.zeros((2, DEPTH, B, H), np.float32)
    states_c = np.zeros((2, DEPTH, B, H), np.float32)
    for c in range(8):
        d, s = c // 4, c % 4
        rows = slice(32 * s, 32 * s + 32)
        r = res.results[c]
        sq = r["seq"].reshape(D, BC, T).transpose(1, 2, 0)  # [b, t, feat]
        hs = r["h_out"].transpose(0, 2, 1, 3).reshape(DEPTH, D, BC).transpose(0, 2, 1)  # [l, b, feat]
        cs = r["c_out"].transpose(0, 2, 1, 3).reshape(DEPTH, D, BC).transpose(0, 2, 1)
        if d == 0:
            out[rows, :, :H] = sq
        else:
            out[rows, :, H:] = sq[:, ::-1, :]
        states_h[d, :, rows, :] = hs
        states_c[d, :, rows, :] = cs
    return out, states_h, states_c


# revision 26
# speedup vs baseline: 121585988.0000x; 56334816.0000x over previous
"""Trainium2 Bass kernel for nn_Encoder (inception-conv + bidirectional stacked residual LSTM).

Sharding: 8 cores = 2 directions x 4 batch-quarters (B=32 per core).
Backward cores receive time-reversed tokens from the host (the conv stack is
time-symmetric, so every core runs a pure forward scan); the host reverses the
backward outputs back.

Per-core program (uniform SPMD, direction is data):
  - embedding via one-hot matmul, fused into conv block 1 (transposed layout
    [D-chunk parts, t])
  - 2 inception blocks: K=3/5/7 convs as tap-shifted f32r matmuls accumulated
    in PSUM, tanh (+bias) on ACT, residual add, LayerNorm via ones-matmul
    column reduction
  - 3 layer-serial LSTM scans, B=32: z kept in PSUM windows; zx (f32r,
    N=256) and bias (K=1 matmul) pre-accumulated per 8-step window; per-step
    recurrent h@Wh in bf16; gates i,f,o,g reordered so one sigmoid + one tanh
    covers them; cell state fp32.
"""

import sys

sys.path.insert(0, "/opt/trn_rl_repo")

import numpy as np
import ml_dtypes

import concourse.bass as bass
import concourse.tile as tile
from concourse import bacc, mybir
from concourse.bass_utils import run_bass_kernel_spmd

F32R = mybir.dt.float32r
BF16 = mybir.dt.bfloat16
F32 = mybir.dt.float32

B, T, D, H, V = 128, 512, 256, 256, 64
DEPTH, N_INC = 3, 2
KSIZES = (3, 5, 7)
LN_EPS = 1e-3
BC = 32          # batch rows per core
KC = D // 128    # k chunks (2)
MC = (4 * H) // 128  # m chunks of gate dim (8)
WIN = 16         # scan window (steps per PSUM group; 16x32xfp32 = 1 PSUM bank per m-chunk)
# gate reorder: reference order is i,f,g,o (split of 4H); we use i,f,o,g
GATE_PERM = np.concatenate([np.arange(0, 256), np.arange(256, 512),
                            np.arange(768, 1024), np.arange(512, 768)])


def build_nc(t_len=T, rows=BC, layers=DEPTH, blocks=N_INC, dump_x2=False, do_conv=True):
    nwin = t_len // WIN
    nc = bacc.Bacc("TRN2", target_bir_lowering=False, debug=False,
                   enable_asserts=False, num_devices=8)

    # ---- inputs (float32r decls accept np.float32 arrays) ----
    tok = nc.dram_tensor("tok", [rows, t_len], F32, kind="ExternalInput")
    iota = nc.dram_tensor("iota", [V, 1], F32, kind="ExternalInput")
    e_w = nc.dram_tensor("e_w", [V, D], F32R, kind="ExternalInput")
    # conv weights packed per block: [blocks, 15, KC, MC2, 128] rows of taps
    # stored as [blocks, ntap_total=15, D(=cin), D(=cout)] -> we declare full
    convw = nc.dram_tensor("convw", [blocks, 15, D, D], F32R, kind="ExternalInput")
    convb = nc.dram_tensor("convb", [blocks, 3, D], F32, kind="ExternalInput")
    gamma = nc.dram_tensor("gamma", [blocks, D], F32, kind="ExternalInput")
    beta = nc.dram_tensor("beta", [blocks, D], F32, kind="ExternalInput")
    wx = nc.dram_tensor("wx", [layers, D, 4 * H], F32R, kind="ExternalInput")
    wh = nc.dram_tensor("wh", [layers, H, 4 * H], BF16, kind="ExternalInput")
    bl = nc.dram_tensor("bl", [layers, 4 * H], F32R, kind="ExternalInput")

    # ---- outputs ----
    seq = nc.dram_tensor("seq", [KC, 128, t_len, rows], F32R, kind="ExternalOutput")
    dbg = (nc.dram_tensor("dbg_x2", [KC, 128, rows, t_len], F32R,
                          kind="ExternalOutput") if dump_x2 else None)
    h_out = nc.dram_tensor("h_out", [layers, 128, KC, rows], F32, kind="ExternalOutput")
    c_out = nc.dram_tensor("c_out", [layers, 128, KC, rows], F32, kind="ExternalOutput")

    TAP_OFF = {3: 0, 5: 3, 7: 8}  # tap index offset inside the packed 15

    with tile.TileContext(nc) as tc:
        with tc.tile_pool(name="dram", bufs=1, space="DRAM") as dpool, \
             tc.tile_pool(name="consts", bufs=1) as cpool:
            # conv/scan streams in DRAM, layout [KC, 128, t, rows]
            # stream0 (conv output) is [KC,128,rows,t] (t contiguous, matches
            # per-row conv writes); cur streams are [KC,128,t,rows] (batch
            # contiguous, matches per-window scan DMAs)
            streams = [dpool.tile([KC, 128, rows, t_len] if si == 0 else
                                  [KC, 128, t_len, rows], F32R, name=f"stream{si}")
                       for si in range(layers)]
            x2 = streams[0]

            iota_sb = cpool.tile([V, 1], F32)
            nc.sync.dma_start(out=iota_sb, in_=iota[:, :])
            e_sb = cpool.tile([V, D], F32R)
            nc.sync.dma_start(out=e_sb, in_=e_w[:, :])
            ones_sb = cpool.tile([1, WIN * rows], F32R)
            nc.vector.memset(ones_sb.bitcast(F32), 1.0)
            eps_sb = cpool.tile([1, 1], F32)
            nc.vector.memset(eps_sb, LN_EPS)
            ones128 = cpool.tile([128, 1], F32R)
            nc.vector.memset(ones128.bitcast(F32), 1.0)
            gb_sb = cpool.tile([128, blocks, 2, KC], F32)  # gamma/beta chunks
            for i in range(blocks):
                for k in range(KC):
                    nc.sync.dma_start(out=gb_sb[:, i, 0, k:k+1],
                                      in_=gamma[i, k * 128:(k + 1) * 128])
                    nc.sync.dma_start(out=gb_sb[:, i, 1, k:k+1],
                                      in_=beta[i, k * 128:(k + 1) * 128])
            cb_sb = cpool.tile([128, blocks, 3, KC], F32)  # conv bias chunks
            for i in range(blocks):
                for j in range(3):
                    for k in range(KC):
                        nc.sync.dma_start(out=cb_sb[:, i, j, k:k+1],
                                          in_=convb[i, j, k * 128:(k + 1) * 128])

            # ---------------- conv phase ----------------
            with tc.tile_pool(name="cw", bufs=1) as wpool, \
                 tc.tile_pool(name="cd", bufs=3, space="DRAM") as dspool, \
                 tc.tile_pool(name="cx", bufs=2) as xpool, \
                 tc.tile_pool(name="cf", bufs=2) as fpool, \
                 tc.tile_pool(name="ct", bufs=2) as tpool, \
                 tc.tile_pool(name="cs", bufs=2) as spool, \
                 tc.tile_pool(name="cps", bufs=2, space="PSUM") as cps, \
                 tc.tile_pool(name="sps", bufs=2, space="PSUM") as sps:
                w_sb = wpool.tile([128, blocks, 15, KC, KC, 128], F32R)
                for i in range(blocks):
                    for tap in range(15):
                        for k in range(KC):
                            # convw[i, tap, cin, cout]: lhsT slice [cin128, cout128]
                            for m in range(KC):
                                nc.sync.dma_start(
                                    out=w_sb[:, i, tap, k, m, :],
                                    in_=convw[i, tap, k * 128:(k + 1) * 128,
                                              m * 128:(m + 1) * 128])

                for b in range(rows):
                    # one-hot: [V, t]
                    ohf = xpool.tile([V, t_len], F32, tag="ohf")
                    tok_b = bass.AP(tensor=tok.ap().tensor, offset=b * t_len,
                                    ap=[[0, V], [1, t_len]])
                    nc.sync.dma_start(out=ohf, in_=tok_b)
                    oh = xpool.tile([V, t_len], F32R, tag="oh")
                    nc.vector.tensor_scalar(
                        out=oh, in0=ohf, scalar1=iota_sb[:V, :],
                        scalar2=None, op0=mybir.AluOpType.is_equal)

                    # xpad [128, KC, t+6] with zero edges
                    xpad = xpool.tile([128, KC, t_len + 6], F32R, tag="xpad")
                    nc.vector.memset(xpad[:, :, 0:3].bitcast(F32), 0.0)
                    nc.vector.memset(xpad[:, :, t_len + 3:t_len + 6].bitcast(F32), 0.0)
                    for m in range(KC):
                        pe = cps.tile([128, t_len], F32, tag="embps")
                        nc.tensor.matmul(pe[:, :], e_sb[:, m * 128:(m + 1) * 128],
                                         oh[:, :], start=True, stop=True)
                        nc.scalar.activation(xpad[:, m, 3:3 + t_len], pe[:, :],
                                             mybir.ActivationFunctionType.Copy)

                    if not do_conv:
                        for m in range(KC):
                            nc.sync.dma_start(out=x2[m, :, b, :],
                                              in_=xpad[:, m, 3:3 + t_len])
                    for i in (range(blocks) if do_conv else ()):
                        feats = fpool.tile([128, KC, t_len], F32R, tag="feats")
                        for j, ks in enumerate(KSIZES):
                            for m in range(KC):
                                pj = cps.tile([128, t_len], F32, tag="convps")
                                nmm = ks * KC
                                cnt = 0
                                for tap in range(ks):
                                    off = 3 + tap - ks // 2
                                    for k in range(KC):
                                        nc.tensor.matmul(
                                            pj[:, :],
                                            w_sb[:, i, TAP_OFF[ks] + tap, k, m, :],
                                            xpad[:, k, off:off + t_len],
                                            start=(cnt == 0), stop=(cnt == nmm - 1))
                                        cnt += 1
                                # tanh(conv + bias) -> accumulate into feats
                                if j == 0:
                                    nc.scalar.activation(
                                        feats[:, m, :], pj[:, :],
                                        mybir.ActivationFunctionType.Tanh,
                                        bias=cb_sb[:, i, j, m:m+1])
                                else:
                                    tj = tpool.tile([128, t_len], F32R, tag="tj")
                                    nc.scalar.activation(
                                        tj[:, :], pj[:, :],
                                        mybir.ActivationFunctionType.Tanh,
                                        bias=cb_sb[:, i, j, m:m+1])
                                    nc.vector.tensor_add(feats[:, m, :],
                                                         feats[:, m, :], tj[:, :])
                        # residual add x
                        for m in range(KC):
                            nc.vector.tensor_add(feats[:, m, :], feats[:, m, :],
                                                 xpad[:, m, 3:3 + t_len])
                        # LayerNorm over D via ones-matmul
                        mu_ps = sps.tile([1, t_len], F32, tag="mups")
                        sq = tpool.tile([128, t_len], F32R, tag="sq")
                        sq_ps = sps.tile([1, t_len], F32, tag="sqps")
                        for m in range(KC):
                            nc.tensor.matmul(mu_ps[:, :], ones128[:, :],
                                             feats[:, m, :], start=(m == 0),
                                             stop=(m == KC - 1))
                        for m in range(KC):
                            nc.scalar.square(sq[:, :], feats[:, m, :])
                            nc.tensor.matmul(sq_ps[:, :], ones128[:, :], sq[:, :],
                                             start=(m == 0), stop=(m == KC - 1))
                        stat = spool.tile([1, 4, t_len], F32, tag="stat")
                        nc.vector.tensor_scalar_mul(stat[:, 0, :], mu_ps[:, :],
                                                    1.0 / D)   # mean
                        nc.vector.tensor_scalar_mul(stat[:, 1, :], sq_ps[:, :],
                                                    1.0 / D)   # E[x^2]
                        nc.vector.tensor_mul(stat[:, 2, :], stat[:, 0, :],
                                             stat[:, 0, :])    # mean^2
                        nc.vector.tensor_sub(stat[:, 1, :], stat[:, 1, :],
                                             stat[:, 2, :])    # var
                        nc.scalar.activation(stat[:, 1, :], stat[:, 1, :],
                                             mybir.ActivationFunctionType.Sqrt,
                                             bias=eps_sb[:, :])
                        nc.vector.reciprocal(stat[:, 2, :], stat[:, 1, :])  # rstd
                        # broadcast mean/rstd across partitions (via DRAM)
                        stat_d = dspool.tile([2, t_len], F32, tag="stat_d")
                        nc.sync.dma_start(out=stat_d[0:1, :], in_=stat[:, 0, :])
                        nc.sync.dma_start(out=stat_d[1:2, :], in_=stat[:, 2, :])
                        bc2 = spool.tile([128, 2, t_len], F32, tag="bc2")
                        sd_ap = bass.AP(tensor=stat_d.tensor, offset=stat_d.offset,
                                        ap=[[0, 128], [t_len, 2], [1, t_len]])
                        nc.sync.dma_start(out=bc2[:, :, :], in_=sd_ap)
                        is_last = (i == blocks - 1)
                        for m in range(KC):
                            nc.vector.scalar_tensor_tensor(
                                out=feats[:, m, :], in0=feats[:, m, :],
                                scalar=1.0, in1=bc2[:, 0, :],
                                op0=mybir.AluOpType.mult,
                                op1=mybir.AluOpType.subtract)
                            nc.vector.tensor_mul(feats[:, m, :], feats[:, m, :],
                                                 bc2[:, 1, :])
                            dst = xpad[:, m, 3:3 + t_len]
                            nc.vector.tensor_scalar(
                                out=dst, in0=feats[:, m, :],
                                scalar1=gb_sb[:, i, 0, m:m+1], scalar2=gb_sb[:, i, 1, m:m+1],
                                op0=mybir.AluOpType.mult, op1=mybir.AluOpType.add)
                        if is_last:
                            for m in range(KC):
                                nc.sync.dma_start(out=x2[m, :, b, :],
                                                  in_=xpad[:, m, 3:3 + t_len])

            if dump_x2:
                for k in range(KC):
                    n_el = 128 * rows * t_len
                    src_ap = bass.AP(tensor=x2.tensor, offset=x2.offset + k * n_el,
                                     ap=[[rows * t_len, 128], [1, rows * t_len]])
                    dst_ap = bass.AP(tensor=dbg.ap().tensor, offset=k * n_el,
                                     ap=[[rows * t_len, 128], [1, rows * t_len]])
                    nc.sync.dma_start(out=dst_ap, in_=src_ap)

            # ---------------- scan phase ----------------
            with tc.tile_pool(name="sw", bufs=1) as swpool, \
                 tc.tile_pool(name="st", bufs=5) as stpool, \
                 tc.tile_pool(name="sst", bufs=1) as sstpool, \
                 tc.tile_pool(name="zps", bufs=1, space="PSUM") as zps:
                wx_sb = swpool.tile([128, layers, KC, MC, 128], F32R)
                wh_sb = swpool.tile([128, layers, KC, MC, 128], BF16)
                for l in range(layers):
                    for k in range(KC):
                        for m in range(MC):
                            nc.sync.dma_start(
                                out=wx_sb[:, l, k, m, :],
                                in_=wx[l, k * 128:(k + 1) * 128,
                                       m * 128:(m + 1) * 128])
                            nc.sync.dma_start(
                                out=wh_sb[:, l, k, m, :],
                                in_=wh[l, k * 128:(k + 1) * 128,
                                       m * 128:(m + 1) * 128])
                bl_sb = swpool.tile([1, layers, MC, 128], F32R)
                for l in range(layers):
                    nc.sync.dma_start(out=bl_sb[0:1, l, :, :], in_=bl[l:l + 1, :])

                for l in range(layers):
                    src = streams[l]
                    dst = seq.ap() if l == layers - 1 else streams[l + 1]
                    cst = sstpool.tile([128, KC, rows], F32, tag="c_state")
                    nc.vector.memset(cst, 0.0)
                    h_prev = sstpool.tile([128, KC, rows], BF16, tag="h_init")
                    nc.vector.memset(h_prev, 0.0)

                    for w in range(nwin):
                        t0 = w * WIN
                        if l == 0:
                            cin = stpool.tile([128, KC, rows, WIN], F32R, tag="cin0")
                            for k in range(KC):
                                nc.sync.dma_start(out=cin[:, k, :, :],
                                                  in_=src[k, :, :, t0:t0 + WIN])
                        else:
                            cin = stpool.tile([128, KC, WIN, rows], F32R, tag="cin")
                            for k in range(KC):
                                nc.sync.dma_start(out=cin[:, k, :, :],
                                                  in_=src[k, :, t0:t0 + WIN, :])
                        zw = zps.tile([128, MC, WIN, rows], F32, tag="zw")
                        for m in range(MC):
                            for k in range(KC):
                                rhs = cin[:, k, :, :]
                                if l == 0:
                                    rhs = rhs.rearrange("p b t -> p t b")
                                nc.tensor.matmul(
                                    zw[:, m, :, :], wx_sb[:, l, k, m, :],
                                    rhs, start=(k == 0), stop=False,
                                    skip_group_check=True)
                            nc.tensor.matmul(
                                zw[:, m, :, :], bl_sb[0:1, l, m, :], ones_sb[:, :],
                                start=False, stop=False, skip_group_check=True)
                        cout = stpool.tile([128, KC, WIN, rows], F32R, tag="cout")
                        for s in range(WIN):
                            # i,f,g gate chunks first so the cell-state chain
                            # can start while the o-gate matmuls still run
                            for m in (0, 1, 2, 3, 6, 7, 4, 5):
                                for k in range(KC):
                                    nc.tensor.matmul(
                                        zw[:, m, s, :], wh_sb[:, l, k, m, :],
                                        h_prev[:, k, :], start=False,
                                        stop=(k == KC - 1), skip_group_check=True)
                            sig = stpool.tile([128, 6, rows], BF16, tag="sig")
                            nc.scalar.activation(sig[:, 0:4, :], zw[:, 0:4, s, :],
                                                 mybir.ActivationFunctionType.Sigmoid)
                            tg = stpool.tile([128, KC, rows], BF16, tag="tg")
                            nc.scalar.activation(tg[:, :, :], zw[:, 6:8, s, :],
                                                 mybir.ActivationFunctionType.Tanh)
                            nc.scalar.activation(sig[:, 4:6, :], zw[:, 4:6, s, :],
                                                 mybir.ActivationFunctionType.Sigmoid)
                            u = stpool.tile([128, KC, rows], F32, tag="u")
                            nc.vector.tensor_mul(u[:, :, :], sig[:, 0:2, :],
                                                 tg[:, :, :])
                            mc_t = stpool.tile([128, KC, rows], F32, tag="mc")
                            nc.vector.tensor_mul(mc_t[:, :, :], cst[:, :, :],
                                                 sig[:, 2:4, :])
                            nc.vector.tensor_add(cst[:, :, :], mc_t[:, :, :],
                                                 u[:, :, :])
                            tc_t = stpool.tile([128, KC, rows], BF16, tag="tc")
                            nc.scalar.activation(tc_t[:, :, :], cst[:, :, :],
                                                 mybir.ActivationFunctionType.Tanh)
                            h_new = stpool.tile([128, KC, rows], BF16, tag="h")
                            nc.vector.tensor_mul(h_new[:, :, :], sig[:, 4:6, :],
                                                 tc_t[:, :, :])
                            cin_s = (cin[:, :, :, s] if l == 0
                                     else cin[:, :, s, :])
                            nc.vector.tensor_add(cout[:, :, s, :], cin_s,
                                                 h_new[:, :, :])
                            h_prev = h_new
                        for k in range(KC):
                            dst_ap = bass.AP(
                                tensor=dst.tensor,
                                offset=dst.offset + k * 128 * rows * t_len
                                + (t0 + s) * 0 + t0 * rows,
                                ap=[[t_len * rows, 128], [rows, WIN], [1, rows]])
                            nc.sync.dma_start(out=dst_ap, in_=cout[:, k, :, :])
                    # final states
                    hf = stpool.tile([128, KC, rows], F32, tag="hf")
                    nc.vector.tensor_copy(hf[:, :, :], h_prev[:, :, :])
                    nc.sync.dma_start(out=h_out[l, :, :, :], in_=hf[:, :, :])
                    nc.sync.dma_start(out=c_out[l, :, :, :], in_=cst[:, :, :])

    nc.compile()
    return nc


_NC_CACHE = {}


def _get_nc():
    if "nc" not in _NC_CACHE:
        _NC_CACHE["nc"] = build_nc()
    return _NC_CACHE["nc"]


def make_in_maps(tokens, E, W3, W5, W7, bconv, gamma, beta, Wx, Wh, blstm):
    tokens = np.asarray(tokens)
    E = np.asarray(E, np.float32)
    W3 = np.asarray(W3, np.float32)
    W5 = np.asarray(W5, np.float32)
    W7 = np.asarray(W7, np.float32)
    bconv = np.asarray(bconv, np.float32)
    gamma_np = np.asarray(gamma, np.float32)
    beta_np = np.asarray(beta, np.float32)
    Wx = np.asarray(Wx, np.float32)
    Wh = np.asarray(Wh, np.float32)
    blstm = np.asarray(blstm, np.float32)

    # packed conv weights [blocks, 15, cin, cout] in tap order K3,K5,K7.
    # Backward cores run on time-reversed tokens; reverse(conv(x, W)) =
    # conv(reverse(x), flip(W, taps)), so they get tap-flipped weights.
    convw_f = np.concatenate([W3, W5, W7], axis=1)
    convw_b = np.concatenate([W3[:, ::-1], W5[:, ::-1], W7[:, ::-1]], axis=1)
    convw_b = np.ascontiguousarray(convw_b)
    iota = np.arange(V, dtype=np.float32).reshape(V, 1)

    in_maps = []
    for c in range(8):
        d = c // 4            # 0 = forward, 1 = backward
        s = c % 4
        rows = slice(32 * s, 32 * s + 32)
        tk = tokens[rows].astype(np.float32)
        if d == 1:
            tk = tk[:, ::-1]
        in_maps.append({
            "tok": np.ascontiguousarray(tk),
            "iota": iota,
            "e_w": E,
            "convw": convw_f if d == 0 else convw_b,
            "convb": bconv,
            "gamma": gamma_np,
            "beta": beta_np,
            "wx": np.ascontiguousarray(Wx[d][:, :, GATE_PERM]),
            "wh": np.ascontiguousarray(Wh[d][:, :, GATE_PERM]).astype(
                ml_dtypes.bfloat16),
            "bl": np.ascontiguousarray(blstm[d][:, GATE_PERM]),
        })

    return in_maps


def kernel(**inputs):
    global _last_in_maps
    in_maps = make_in_maps(**inputs)
    _last_in_maps = in_maps
    nc = _get_nc()
    res = run_bass_kernel_spmd(nc, in_maps, core_ids=list(range(8)))

    out = np.zeros((B, T, 2 * H), np.float32)
    states_h = np.zeros((2, DEPTH, B, H), np.float32)
    states_c = np.zeros((2, DEPTH, B, H), np.float32)
    for c in range(8):
        d, s = c // 4, c % 4
        rows = slice(32 * s, 32 * s + 32)
        r = res.results[c]
        sq = r["seq"].reshape(D, T, BC).transpose(2, 1, 0)  # [b, t, feat]
        hs = r["h_out"].transpose(0, 2, 1, 3).reshape(DEPTH, D, BC).transpose(0, 2, 1)  # [l, b, feat]
        cs = r["c_out"].transpose(0, 2, 1, 3).reshape(DEPTH, D, BC).transpose(0, 2, 1)
        if d == 0:
            out[rows, :, :H] = sq
        else:
            out[rows, :, H:] = sq[:, ::-1, :]
        states_h[d, :, rows, :] = hs
        states_c[d, :, rows, :] = cs
    return out, states_h, states_c
